# revision 1
# baseline (speedup 1.0000x reference)
"""MDTA (Restormer transposed channel attention) Trainium2 kernel.

Data-parallel over batch: 8 batch elements -> 8 NeuronCores, one each.

Per-core algorithm (matmuls take bf16 inputs, accumulate fp32 in PSUM):
  - x is uploaded as bf16 and DMA'd into a vertically-shifted padded
    "x2 stack" resident in SBUF: row (ty*48+c), ty in {0,1}, at free pos q
    holds x_pad[c, q + (ty-1)*258] over a 258-wide zero-padded image
    layout.  The third vertical tap reads the ty=0 rows at +2*258.
    Vertical conv taps thus come from partition placement / big free
    offsets, horizontal taps from small free-dim AP offsets, so the 3x3
    depthwise conv fuses into dense matmuls against the same buffer.
  - Phase 1: fused (1x1 conv + depthwise 3x3) for q,k: 6 matmuls per
    512-pixel chunk -> [96, 512] PSUM.  Chunks are transposed on the PE
    and accumulated into Gram matrices holding all per-head q.k^T blocks
    plus squared L2 norms on the diagonals.  The per-chunk post-processing
    (transpose / Gram) is emitted 1-2 chunks behind the conv so the PE
    never waits on the ACT/DVE copies.
  - Attention: logits scaled by 1/max(||q||,eps) * 1/max(||k||,eps) *
    temperature, masked block-diagonal softmax over 6-wide head blocks.
  - P2 = proj_w @ attn, then per-tap phase-2 weights C_tap^T = V_ext^T @
    P2^T are built on device (V_ext bakes the v-path 1x1 conv, depthwise
    weights and stack placement).
  - Phase 2: final output = sum over taps of C^T stacks applied to the
    same x2 stack; 6 matmuls per chunk -> [48, 512] -> DMA out.
"""

import functools
import sys

_build_conv_n = 512
_BUFS = dict(qkp=3, tp=2, wp=3, op=4)

if "/opt/trn_rl_repo" not in sys.path:
    sys.path.insert(0, "/opt/trn_rl_repo")

import ml_dtypes
import numpy as np

import concourse.bass as bass
import concourse.tile as tile
from concourse import bacc, mybir
from concourse import bass_utils

BF16 = ml_dtypes.bfloat16
F32 = np.float32

B, C, H, W = 8, 48, 256, 256
HEADS, HD = 8, 6
PW = W + 2                 # padded row width
PF = PW * (H + 2)          # padded flat image size
X2F = PF + 2 * PW + 4      # x2 buffer free size (+2 rows for the ty=2 read)
NCH = (H * W) // 512       # 128 chunks of 512 pixels (2 image rows)
EPS = 1e-12

bf = mybir.dt.bfloat16
f32 = mybir.dt.float32


def _win(t, p0, p1, ci, dx, extra=0):
    """rhs window: 512 output pixels of chunk ci at horiz tap dx."""
    off = (2 * ci + 1) * PW + dx + extra
    return t[p0:p1, off:off + 2 * PW].rearrange(
        "p (r w) -> p r w", w=PW)[:, :, 0:W]


@functools.cache
def _build(repeat=1, upto=3, xpose="dma2", p1_post=True, fillsplit=8,
           fillengines=("sync", "scalar"), gmode="g2", cpeng="dve"):
    # upto: 1 = x2 fill only, 2 = + phase 1, 3 = full kernel (bisect aid)
    # xpose: "pe" = TensorE transpose via PSUM; "dma" = xbar DMA transpose
    # p1_post=False: phase-1 convs only (no copy/transpose/Gram) - bisect aid
    # fillsplit/fillengines: x2 interior DMA chunking and issuing engines
    conv_n = _build_conv_n  # timing aid: shrink conv rhs N (breaks output)
    nc = bacc.Bacc("TRN2", target_bir_lowering=False, debug=False)

    # xb rows are host-padded to 258 ([0, row, 0]) so the x2 interior fill
    # is a fully contiguous DMA and the pad columns need no memset.
    xb = nc.dram_tensor("xb", [C, H * PW], bf, kind="ExternalInput").ap()
    w01_d = nc.dram_tensor("w01", [96, 3 * 96], bf, kind="ExternalInput").ap()
    w2_d = nc.dram_tensor("w2", [48, 3 * 96], bf, kind="ExternalInput").ap()
    vA_d = nc.dram_tensor("vA", [48, 6 * 96], bf, kind="ExternalInput").ap()
    vB_d = nc.dram_tensor("vB", [48, 3 * 48], bf, kind="ExternalInput").ap()
    projT_d = nc.dram_tensor("projT", [48, 48], bf, kind="ExternalInput").ap()
    id96_d = nc.dram_tensor("id96", [96, 96], bf, kind="ExternalInput").ap()
    eye48_d = nc.dram_tensor("eye48", [48, 48], f32, kind="ExternalInput").ap()
    mask_d = nc.dram_tensor("maskbd", [48, 48], f32, kind="ExternalInput").ap()
    temp_d = nc.dram_tensor("temppc", [48, 1], f32, kind="ExternalInput").ap()
    y = nc.dram_tensor("y", [C, H * W], f32, kind="ExternalOutput").ap()

    with tile.TileContext(nc) as tc:
        with (
            tc.tile_pool(name="const", bufs=1) as cpool,
            tc.tile_pool(name="x2", bufs=1) as x2pool,
            tc.tile_pool(name="work", bufs=_BUFS["wp"]) as wpool,
            tc.tile_pool(name="small", bufs=1) as spool,
        ):
            # ---- constants to SBUF ----
            w01 = cpool.tile([96, 3 * 96], bf)
            w2 = cpool.tile([48, 3 * 96], bf)
            vA = cpool.tile([48, 6 * 96], bf)
            vB = cpool.tile([48, 3 * 48], bf)
            projT = cpool.tile([48, 48], bf)
            id96 = cpool.tile([96, 96], bf)
            eye48 = cpool.tile([48, 48], f32)
            maskbd = cpool.tile([48, 48], f32)
            temppc = cpool.tile([48, 1], f32)
            for dst, src in [(w01, w01_d), (w2, w2_d), (vA, vA_d),
                             (vB, vB_d), (projT, projT_d), (id96, id96_d),
                             (eye48, eye48_d), (maskbd, mask_d),
                             (temppc, temp_d)]:
                nc.sync.dma_start(dst[:], src[:])

            x2 = x2pool.tile([96, X2F], bf)

            for _rep in range(repeat):
                # ---- x2 stack: zero borders, DMA padded interiors ----
                # ty block tb at partitions 48*tb holds x_pad rows starting
                # at q-row (2-tb); rows are host-padded so the interior copy
                # is contiguous.  Top/bottom pad rows are memset (memsets
                # conservatively cover both blocks; DMAs overwrite after).
                nc.vector.memset(x2[0:96, 0:2 * PW], 0.0)        # top rows
                nc.vector.memset(x2[0:96, (H + 1) * PW:X2F], 0.0)  # bottom
                part = (H // fillsplit) * PW
                engs = [getattr(nc, e) for e in fillengines]
                di = 0
                for tb in range(2):
                    o0 = (2 - tb) * PW
                    for hh in range(fillsplit):
                        engs[di % len(engs)].dma_start(
                            x2[48 * tb:48 * tb + 48,
                               o0 + hh * part:o0 + (hh + 1) * part],
                            xb[:, hh * part:(hh + 1) * part])
                        di += 1

                if upto < 2:
                    continue

                # ---- phase 1: qk fused conv + Gram (SW-pipelined) ----
                with (
                    tc.tile_pool(name="psG", bufs=1, space="PSUM") as gpool,
                    tc.tile_pool(name="psqk", bufs=_BUFS["qkp"], space="PSUM") as qkp,
                    tc.tile_pool(name="pst", bufs=_BUFS["tp"], space="PSUM") as tpool,
                ):
                    gw = 96 if gmode == "g2" else 48
                    G1 = gpool.tile([48, gw], f32)  # k-rows: [k.q | k.k]
                    G2 = (gpool.tile([48, 48], f32, name="G2", tag="G2")
                          if gmode == "g2" else None)  # q.q
                    ssqacc = (spool.tile([96, NCH], f32, name="ssqacc")
                              if gmode == "sq" else None)
                    qk_sbs, qkTs = {}, {}

                    def conv_qk(ci):
                        qk_ps = qkp.tile([96, 512], f32)
                        for dx in range(3):
                            nc.tensor.matmul(
                                qk_ps[:, 0:conv_n], w01[:, dx * 96:(dx + 1) * 96],
                                _win(x2, 0, 96, ci, dx)[:, 0:1, 0:conv_n]
                                if conv_n < 512 else _win(x2, 0, 96, ci, dx),
                                start=(dx == 0), stop=False)
                            nc.tensor.matmul(
                                qk_ps[:, 0:conv_n], w2[:, dx * 96:(dx + 1) * 96],
                                _win(x2, 0, 48, ci, dx, extra=2 * PW)[:, 0:1, 0:conv_n]
                                if conv_n < 512 else
                                _win(x2, 0, 48, ci, dx, extra=2 * PW),
                                start=False, stop=(dx == 2))
                        if not p1_post:
                            return
                        cp, half = ci // 2, ci % 2
                        if half == 0:
                            qk_sb = wpool.tile([96, 1024], bf, tag="qksb")
                            qk_sbs[cp] = qk_sb
                        else:
                            qk_sb = qk_sbs[cp]
                        dst = qk_sb[:, half * 512:(half + 1) * 512]
                        if cpeng == "dve" or (cpeng == "alt" and ci % 2):
                            nc.vector.tensor_copy(dst, qk_ps[:])
                        else:
                            nc.scalar.copy(dst, qk_ps[:])

                    def transp(cp):
                        qk_sb = qk_sbs.pop(cp)
                        qkT = wpool.tile([128, 768], bf, tag="qkT")
                        if xpose == "dma":
                            for j in range(4):
                                nc.sync.dma_start_transpose(
                                    qkT[:, j * 96:(j + 1) * 96],
                                    qk_sb[:, j * 128:(j + 1) * 128])
                        elif xpose == "dma2":
                            for j in range(8):
                                eng = nc.sync if j % 2 == 0 else nc.scalar
                                eng.dma_start_transpose(
                                    qkT[:, j * 96:(j + 1) * 96],
                                    qk_sb[:, j * 128:(j + 1) * 128])
                        elif xpose == "mix":
                            nc.sync.dma_start_transpose(
                                qkT[:, 0:96], qk_sb[:, 0:128])
                            nc.scalar.dma_start_transpose(
                                qkT[:, 96:192], qk_sb[:, 128:256])
                            ps_t = tpool.tile([128, 192], bf)
                            for j in (2, 3):
                                nc.tensor.transpose(
                                    ps_t[:, (j - 2) * 96:(j - 1) * 96],
                                    qk_sb[:, j * 128:(j + 1) * 128], id96[:])
                            nc.vector.tensor_copy(qkT[:, 192:384], ps_t[:])
                        else:
                            ps_t = tpool.tile([128, 384], bf)
                            for j in range(4):
                                nc.tensor.transpose(
                                    ps_t[:, j * 96:(j + 1) * 96],
                                    qk_sb[:, j * 128:(j + 1) * 128], id96[:])
                            nc.vector.tensor_copy(qkT[:], ps_t[:])
                        qkTs[cp] = qkT

                    def gram(cp):
                        qkT = qkTs.pop(cp)
                        for j in range(8):
                            first = (cp == 0 and j == 0)
                            last = (cp == NCH // 2 - 1 and j == 7)
                            nc.tensor.matmul(
                                G1[:], qkT[:, j * 96 + 48:j * 96 + 96],
                                qkT[:, j * 96:j * 96 + gw],
                                start=first, stop=last)
                            if gmode == "g2":
                                nc.tensor.matmul(
                                    G2[:], qkT[:, j * 96:j * 96 + 48],
                                    qkT[:, j * 96:j * 96 + 48],
                                    start=first, stop=last)

                    if p1_post:
                        for cp in range(NCH // 2):
                            conv_qk(2 * cp)
                            conv_qk(2 * cp + 1)
                            if cp >= 1:
                                transp(cp - 1)
                            if cp >= 2:
                                gram(cp - 2)
                        transp(NCH // 2 - 1)
                        gram(NCH // 2 - 2)
                        gram(NCH // 2 - 1)
                    else:
                        for ci in range(NCH):
                            conv_qk(ci)
                        nc.tensor.matmul(
                            G1[:], w01[:, 0:48],
                            _win(x2, 0, 96, 0, 0)[:, 0:1, 0:48],
                            start=True, stop=True)

                    Gs1 = spool.tile([48, 48], f32)
                    nc.vector.tensor_copy(Gs1[:], G1[:, 0:48])
                    ssqk2 = spool.tile([48, 1], f32)
                    ssqq2 = spool.tile([48, 1], f32)
                    if gmode == "g2":
                        gd = spool.tile([48, 48], f32)
                        nc.vector.tensor_mul(gd[:], G1[:, 48:96], eye48[:])
                        nc.vector.tensor_reduce(
                            ssqk2[:], gd[:], axis=mybir.AxisListType.X,
                            op=mybir.AluOpType.add)
                        gd2 = spool.tile([48, 48], f32)
                        nc.vector.tensor_mul(gd2[:], G2[:], eye48[:])
                        nc.vector.tensor_reduce(
                            ssqq2[:], gd2[:], axis=mybir.AxisListType.X,
                            op=mybir.AluOpType.add)

                if upto < 3:
                    continue

                # ---- attention (tiny, partitions 0-47) ----
                with tc.tile_pool(name="psS", bufs=1, space="PSUM") as spp:
                    nk = spool.tile([48, 1], f32)
                    nq = spool.tile([48, 1], f32)
                    invk = spool.tile([48, 1], f32)
                    invq = spool.tile([48, 1], f32)
                    if gmode == "sq":
                        ssqf = spool.tile([96, 1], f32)
                        nc.vector.tensor_reduce(
                            ssqf[:], ssqacc[:], axis=mybir.AxisListType.X,
                            op=mybir.AluOpType.add)
                        ssqk = spool.tile([48, 1], f32)
                        nc.sync.dma_start(ssqk[:], ssqf[48:96, :])
                        nc.scalar.sqrt(nk[:], ssqk[:])
                        nc.scalar.sqrt(nq[:], ssqf[0:48, :])
                    else:
                        nc.scalar.sqrt(nk[:], ssqk2[:])
                        nc.scalar.sqrt(nq[:], ssqq2[:])
                    nc.vector.tensor_scalar_max(nk[:], nk[:], EPS)
                    nc.vector.tensor_scalar_max(nq[:], nq[:], EPS)
                    nc.vector.reciprocal(invk[:], nk[:])
                    nc.vector.reciprocal(invq[:], nq[:])

                    # logits^T scaled by inv_k (rows = k-channels e)
                    m1 = spool.tile([48, 48], f32)
                    nc.vector.tensor_scalar(
                        m1[:], Gs1[:], invk[:], None,
                        op0=mybir.AluOpType.mult)
                    m1T = spp.tile([48, 48], f32, tag="m1T")
                    nc.tensor.transpose(m1T[:], m1[:], eye48[:])
                    L = spool.tile([48, 48], f32)
                    nc.vector.tensor_scalar(
                        L[:], m1T[:], invq[:], temppc[:],
                        op0=mybir.AluOpType.mult, op1=mybir.AluOpType.mult)
                    nc.vector.tensor_add(L[:], L[:], maskbd[:])
                    nrm = spool.tile([48, 1], f32)
                    nc.vector.tensor_reduce(
                        nrm[:], L[:], axis=mybir.AxisListType.X,
                        op=mybir.AluOpType.max, negate=True)
                    E = spool.tile([48, 48], f32)
                    rowsum = spool.tile([48, 1], f32)
                    nc.scalar.activation(
                        E[:], L[:], mybir.ActivationFunctionType.Exp,
                        bias=nrm[:], scale=1.0, accum_out=rowsum[:])
                    invs = spool.tile([48, 1], f32)
                    nc.vector.reciprocal(invs[:], rowsum[:])
                    attn = spool.tile([48, 48], bf)
                    nc.vector.tensor_scalar(
                        attn[:], E[:], invs[:], None,
                        op0=mybir.AluOpType.mult)

                    # P2^T = attn^T @ proj^T
                    pt_ps = spp.tile([48, 48], f32, tag="ptps")
                    nc.tensor.matmul(pt_ps[:], attn[:], projT[:],
                                     start=True, stop=True)
                    PT = spool.tile([48, 48], bf)
                    nc.vector.tensor_copy(PT[:], pt_ps[:])

                    # phase-2 weight stacks
                    ph2a = spool.tile([96, 3 * 48], bf)
                    ph2b = spool.tile([48, 3 * 48], bf)
                    for dx in range(3):
                        psA = spp.tile([96, 48], f32, tag="psA")
                        for ty in range(2):
                            nc.tensor.matmul(
                                psA[:], vA[:, (dx * 2 + ty) * 96:
                                            (dx * 2 + ty + 1) * 96],
                                PT[:], start=(ty == 0), stop=(ty == 1))
                        nc.vector.tensor_copy(
                            ph2a[:, dx * 48:(dx + 1) * 48], psA[:])
                        psB = spp.tile([48, 48], f32, tag="psB")
                        nc.tensor.matmul(psB[:],
                                         vB[:, dx * 48:(dx + 1) * 48],
                                         PT[:], start=True, stop=True)
                        nc.vector.tensor_copy(
                            ph2b[:, dx * 48:(dx + 1) * 48], psB[:])

                # ---- phase 2: final fused conv + DMA out ----
                with tc.tile_pool(name="psO", bufs=_BUFS["op"], space="PSUM") as opool:
                    for ci in range(NCH):
                        o_ps = opool.tile([48, 512], f32)
                        for dx in range(3):
                            nc.tensor.matmul(
                                o_ps[:], ph2a[:, dx * 48:(dx + 1) * 48],
                                _win(x2, 0, 96, ci, dx),
                                start=(dx == 0), stop=False)
                            nc.tensor.matmul(
                                o_ps[:], ph2b[:, dx * 48:(dx + 1) * 48],
                                _win(x2, 0, 48, ci, dx, extra=2 * PW),
                                start=False, stop=(dx == 2))
                        if ci % 2 == 0:
                            o_sb = wpool.tile([48, 1024], f32, tag="osb")
                        oslc = o_sb[:, (ci % 2) * 512:(ci % 2 + 1) * 512]
                        if cpeng == "dve" or (cpeng == "alt" and ci % 2):
                            nc.vector.tensor_copy(oslc, o_ps[:])
                        else:
                            nc.scalar.copy(oslc, o_ps[:])
                        if ci % 2 == 1:
                            nc.sync.dma_start(
                                y[:, (ci - 1) * 512:(ci + 1) * 512], o_sb[:])

    nc.compile()
    return nc


def _host_weights(qkv_w, dw_w, proj_w, temperature):
    # fused qk weights: w[(ty,c), dx*96+o] = qkv_w[o,c]*dw_w[o,0,ty,dx]
    wfull = np.einsum("oc,otd->tcdo", qkv_w[:96], dw_w[:96, 0]).astype(F32)
    wfull = wfull.reshape(144, 3, 96)          # [(ty,c), dx, o]
    w01 = wfull[:96].reshape(96, 3 * 96).astype(BF16)
    w2 = wfull[96:].reshape(48, 3 * 96).astype(BF16)

    # v-path taps: vA[e, (dx*2+ty)*96 + r] (ty in {0,1}), vB[e, dx*48+c]
    vw = np.einsum("ec,etd->tdec", qkv_w[96:], dw_w[96:, 0]).astype(F32)
    vA = np.zeros((48, 6, 96), F32)
    vB = np.zeros((48, 3, 48), F32)
    for dx in range(3):
        for ty in range(2):
            for c in range(48):
                vA[:, dx * 2 + ty, ty * 48 + c] = vw[ty, dx, :, c]
        vB[:, dx, :] = vw[2, dx]               # [e, c]
    vA = vA.reshape(48, 6 * 96).astype(BF16)
    vB = vB.reshape(48, 3 * 48).astype(BF16)

    projT = proj_w.T.astype(BF16).copy()
    id96 = np.eye(96, dtype=F32).astype(BF16)
    eye48 = np.eye(48, dtype=F32)
    maskbd = np.full((48, 48), -1e9, F32)
    for h in range(HEADS):
        maskbd[h * HD:(h + 1) * HD, h * HD:(h + 1) * HD] = 0.0
    temppc = np.repeat(temperature.reshape(HEADS), HD).reshape(48, 1)
    temppc = temppc.astype(F32)
    return dict(w01=w01, w2=w2, vA=vA, vB=vB, projT=projT, id96=id96,
                eye48=eye48, maskbd=maskbd, temppc=temppc)


def make_in_maps(x, qkv_w, dw_w, proj_w, temperature):
    shared = _host_weights(np.asarray(qkv_w, F32), np.asarray(dw_w, F32),
                           np.asarray(proj_w, F32),
                           np.asarray(temperature, F32))
    xp = np.zeros((B, C, H, PW), F32)
    xp[:, :, :, 1:1 + W] = np.asarray(x, F32).reshape(B, C, H, W)
    xp = xp.reshape(B, C, H * PW).astype(BF16)
    maps = []
    for b in range(B):
        m = dict(shared)
        m["xb"] = xp[b]
        maps.append(m)
    return maps


def kernel(x, qkv_w, dw_w, proj_w, temperature):
    nc = _build()
    in_maps = make_in_maps(x, qkv_w, dw_w, proj_w, temperature)
    res = bass_utils.run_bass_kernel_spmd(nc, in_maps, list(range(B)))
    out = np.stack([res.results[b]["y"].reshape(C, H, W) for b in range(B)])
    return out.astype(np.float32)



# revision 11
# speedup vs baseline: 1.5948x; 1.5948x over previous
"""MDTA (Restormer transposed channel attention) Trainium2 kernel.

Data-parallel over batch: 8 batch elements -> 8 NeuronCores, one each.

Per-core algorithm (matmuls take bf16 inputs, accumulate fp32 in PSUM):
  - x3 [128, X3F] holds three vertically-shifted copies of the padded
    image rows: partitions 0-47 = ty0 (shift 2PW), 48-95 = ty1 (shift
    PW), 96-127 = ty2 for channels 0-31 (shift 0; ch16-31 weight-zero).
    Horizontal taps are free-dim offsets, so each dx tap of the fused
    (1x1 + depthwise 3x3) conv is ONE 128-partition matmul.  The
    remaining (ty2, ch16-47) taps live in xL [96, seg] segments: rows
    (dg*32+c'-16) hold channel c' pre-shifted by dg, so all three
    horizontal taps collapse into ONE extra 96-partition matmul.  4 MMs
    per 512-pixel chunk (vs 6), all with {96,128}-partition
    stationaries and <=4-MM PSUM groups (both measured cliffs on
    TRN2).
  - Phase 1: q,k chunks [96, 512] -> PE-transpose (or DMA) to pixel-
    major [128, 384] -> ONE fused Gram matmul per 128-pixel group:
    G[96,96] += qkT_j.T @ qkT_j gives q.q / q.k / k.k blocks at once.
  - Attention: block-diag softmax over 6-wide head blocks, scaled by
    1/max(||q||,eps) * 1/max(||k||,eps) * temperature.
  - Phase-2 weights C1 [128, 3*48] / CL [48, 48] = (proj @ attn @ v-tap
    weights) built on device; phase 2 applies them to the same x3/xL
    stacks; output DMA'd out as bf16 (upcast on host).
"""

import functools
import sys

_BUFS = dict(qkp=4, tp=2, wp=3, op=4)

if "/opt/trn_rl_repo" not in sys.path:
    sys.path.insert(0, "/opt/trn_rl_repo")

import ml_dtypes
import numpy as np

import concourse.bass as bass
import concourse.tile as tile
from concourse import bacc, mybir
from concourse import bass_utils

BF16 = ml_dtypes.bfloat16
F32 = np.float32

B, C, H, W = 8, 48, 256, 256
HEADS, HD = 8, 6
PW = W + 2                  # padded row width
XBF = H * PW                # host-padded flat image size (66048)
X3F = XBF + PW + 6          # x3 free size (ty0 writes end at 2PW+65790)
NSEG = 8                    # xL segments per image
CSEG = 16                   # chunks per segment
SEGB = 2 * CSEG * PW        # window-base stride per segment (8256)
XLF = SEGB                  # xL tile free size
NCH = (H * W) // 512        # 128 chunks of 512 pixels (2 image rows)
EPS = 1e-12

bf = mybir.dt.bfloat16
f32 = mybir.dt.float32


def _w3(t, ci, dx):
    """x3 window: 512 output pixels of chunk ci at horiz tap dx."""
    off = (2 * ci + 1) * PW + dx
    return t[0:128, off:off + 2 * PW].rearrange(
        "p (r w) -> p r w", w=PW)[:, :, 0:W]


def _wL(t, cl):
    """xL window: 512 output pixels of local chunk cl in segment tile."""
    off = 2 * cl * PW
    return t[0:96, off:off + 2 * PW].rearrange(
        "p (r w) -> p r w", w=PW)[:, :, 0:W]


@functools.cache
def _build(repeat=1, upto=3, xpose="pe", p1_post=True, fillsplit=8,
           cpeng="alt"):
    # upto: 1 = fill only, 2 = + phase 1, 3 = full kernel (bisect aid)
    # xpose: "pe" = TensorE transpose via PSUM; "dma2" = xbar DMA transpose
    # p1_post=False: phase-1 convs only (no copy/transpose/Gram) - bisect aid
    nc = bacc.Bacc("TRN2", target_bir_lowering=False, debug=False)

    # xb rows are host-padded to 258 ([0, row, 0]) so stack fills are
    # contiguous DMAs and pad columns need no per-row memsets.
    xb = nc.dram_tensor("xb", [C, XBF], bf, kind="ExternalInput").ap()
    w1_d = nc.dram_tensor("w1", [128, 3 * 96], bf, kind="ExternalInput").ap()
    wl1_d = nc.dram_tensor("wl1", [96, 96], bf, kind="ExternalInput").ap()
    vA_d = nc.dram_tensor("vA", [48, 6 * 96], bf, kind="ExternalInput").ap()
    vT2_d = nc.dram_tensor("vT2", [48, 3 * 16], bf, kind="ExternalInput").ap()
    vTL_d = nc.dram_tensor("vTL", [48, 3 * 32], bf, kind="ExternalInput").ap()
    projT_d = nc.dram_tensor("projT", [48, 48], bf, kind="ExternalInput").ap()
    id96_d = nc.dram_tensor("id96", [96, 96], bf, kind="ExternalInput").ap()
    eye48_d = nc.dram_tensor("eye48", [48, 48], f32, kind="ExternalInput").ap()
    mask_d = nc.dram_tensor("maskbd", [48, 48], f32, kind="ExternalInput").ap()
    temp_d = nc.dram_tensor("temppc", [48, 1], f32, kind="ExternalInput").ap()
    y = nc.dram_tensor("y", [C, H * W], bf, kind="ExternalOutput").ap()

    with tile.TileContext(nc) as tc:
        with (
            tc.tile_pool(name="const", bufs=1) as cpool,
            tc.tile_pool(name="x3", bufs=1) as x3pool,
            tc.tile_pool(name="xL", bufs=2) as xLpool,
            tc.tile_pool(name="work", bufs=_BUFS["wp"]) as wpool,
            tc.tile_pool(name="small", bufs=1) as spool,
        ):
            # ---- constants to SBUF ----
            w1 = cpool.tile([128, 3 * 96], bf)
            wl1 = cpool.tile([96, 96], bf)
            vA = cpool.tile([48, 6 * 96], bf)
            vT2 = cpool.tile([48, 3 * 16], bf)
            vTL = cpool.tile([48, 3 * 32], bf)
            projT = cpool.tile([48, 48], bf)
            id96 = cpool.tile([96, 96], bf)
            eye48 = cpool.tile([48, 48], f32)
            maskbd = cpool.tile([48, 48], f32)
            temppc = cpool.tile([48, 1], f32)
            for dst, src in [(w1, w1_d), (wl1, wl1_d), (vA, vA_d),
                             (vT2, vT2_d), (vTL, vTL_d), (projT, projT_d),
                             (id96, id96_d), (eye48, eye48_d),
                             (maskbd, mask_d), (temppc, temp_d)]:
                nc.sync.dma_start(dst[:], src[:])

            x3 = x3pool.tile([128, X3F], bf)
            # phase-2 weight stacks (written by the attention section)
            C1 = spool.tile([128, 3 * 48], bf, name="C1")
            CL = spool.tile([96, 96], bf, name="CL")
            nc.vector.memset(C1[96:128, :], 0.0)
            nc.vector.memset(CL[:, 48:96], 0.0)

            def fill_x3():
                # borders: zero top (rows 0-95) and tails (all rows)
                nc.vector.memset(x3[0:96, 0:2 * PW], 0.0)
                nc.vector.memset(x3[0:128, XBF - PW:X3F], 0.0)
                part = XBF // fillsplit
                engs = [nc.sync, nc.scalar]
                di = 0
                # tb=0: ty0 at +2PW (trim tail to stay inside X3F);
                # tb=1: ty1 at +PW; tb=2: ty2 ch0-31 at +0.
                for tb in range(3):
                    o0 = (2 - tb) * PW
                    p0, p1 = 48 * tb, 48 * tb + (32 if tb == 2 else 48)
                    for hh in range(fillsplit):
                        s0, s1 = hh * part, (hh + 1) * part
                        s1 = min(s1, X3F - 6 - o0)
                        engs[di % 2].dma_start(
                            x3[p0:p1, o0 + s0:o0 + s1], xb[0:p1 - p0, s0:s1])
                        di += 1

            def fill_xL(k):
                # segment k: rows (dg*32+cc) = xb[16+cc, base+q+dg]
                xLt = xLpool.tile([96, XLF], bf, tag="xL")
                base = (2 * CSEG * k + 1) * PW
                if k == NSEG - 1:
                    nc.vector.memset(xLt[:, XLF - 2 * PW - 40:XLF], 0.0)
                for dg in range(3):
                    ln = min(XLF, XBF - base - dg)
                    nc.scalar.dma_start(
                        xLt[32 * dg:32 * dg + 32, 0:ln],
                        xb[16:48, base + dg:base + dg + ln])
                return xLt

            for _rep in range(repeat):
                fill_x3()
                if upto < 2:
                    continue

                # ---- phase 1: qk fused conv + Gram (SW-pipelined) ----
                with (
                    tc.tile_pool(name="psG", bufs=1, space="PSUM") as gpool,
                    tc.tile_pool(name="psqk", bufs=_BUFS["qkp"],
                                 space="PSUM") as qkp,
                    tc.tile_pool(name="pst", bufs=_BUFS["tp"],
                                 space="PSUM") as tpool,
                ):
                    G = gpool.tile([48, 512], f32)
                    qk_sbs, qkTs = {}, {}
                    xLt = [None]

                    def conv_qk(ci):
                        if ci % CSEG == 0:
                            xLt[0] = fill_xL(ci // CSEG)
                        qk_ps = qkp.tile([96, 512], f32)
                        for dx in range(3):
                            nc.tensor.matmul(
                                qk_ps[:], w1[:, dx * 96:(dx + 1) * 96],
                                _w3(x3, ci, dx), start=(dx == 0), stop=False)
                        nc.tensor.matmul(
                            qk_ps[:], wl1[:], _wL(xLt[0], ci % CSEG),
                            start=False, stop=True)
                        if not p1_post:
                            return
                        qk_sb = wpool.tile([96, 512], bf, tag="qksb")
                        if cpeng in ("dve",) or (cpeng == "alt" and ci % 2):
                            nc.vector.tensor_copy(qk_sb[:], qk_ps[:])
                        else:
                            nc.scalar.copy(qk_sb[:], qk_ps[:])
                        qk_sbs[ci] = qk_sb

                    def transp(ci):
                        qk_sb = qk_sbs.pop(ci)
                        qkT = wpool.tile([128, 384], bf, tag="qkT")
                        if xpose == "dma2":
                            for j in range(4):
                                eng = nc.sync if j % 2 == 0 else nc.scalar
                                eng.dma_start_transpose(
                                    qkT[:, j * 96:(j + 1) * 96],
                                    qk_sb[:, j * 128:(j + 1) * 128])
                        else:
                            ps_t = tpool.tile([128, 1024], bf)
                            for j in range(4):
                                nc.tensor.transpose(
                                    ps_t[:, j * 96:(j + 1) * 96],
                                    qk_sb[:, j * 128:(j + 1) * 128],
                                    id96[:])
                            if cpeng == "alt" and ci % 2:
                                nc.scalar.copy(qkT[:], ps_t[:, 0:384])
                            else:
                                nc.vector.tensor_copy(qkT[:], ps_t[:, 0:384])
                        qkTs[ci] = qkT

                    def gram(ci):
                        qkT = qkTs.pop(ci)
                        first = ci == 0
                        last = ci == NCH - 1
                        for j in range(4):
                            # k-rows x [q|k]: G[:,0:48]=k.q, G[:,48:96]=k.k
                            nc.tensor.matmul(
                                G[:, 0:96],
                                qkT[:, j * 96 + 48:(j + 1) * 96],
                                qkT[:, j * 96:(j + 1) * 96],
                                start=(first and j == 0),
                                stop=(last and j == 3))
                            # q.q into spare columns of the same bank
                            nc.tensor.matmul(
                                G[:, 96:144],
                                qkT[:, j * 96:j * 96 + 48],
                                qkT[:, j * 96:j * 96 + 48],
                                start=(first and j == 0),
                                stop=(last and j == 3))

                    if p1_post:
                        for ci in range(NCH):
                            conv_qk(ci)
                            if ci >= 1:
                                transp(ci - 1)
                            if ci >= 2:
                                gram(ci - 2)
                        transp(NCH - 1)
                        gram(NCH - 2)
                        gram(NCH - 1)
                    else:
                        for ci in range(NCH):
                            conv_qk(ci)
                        nc.tensor.matmul(
                            G[:, 0:144], w1[0:128, 0:48],
                            _w3(x3, 0, 0)[:, 0:1, 0:144],
                            start=True, stop=True)

                    Gqk = spool.tile([48, 48], f32)
                    nc.vector.tensor_copy(Gqk[:], G[:, 0:48])
                    ssqq2 = spool.tile([48, 1], f32)
                    ssqk2 = spool.tile([48, 1], f32)
                    gd = spool.tile([48, 48], f32)
                    nc.vector.tensor_mul(gd[:], G[:, 96:144], eye48[:])
                    nc.vector.tensor_reduce(
                        ssqq2[:], gd[:], axis=mybir.AxisListType.X,
                        op=mybir.AluOpType.add)
                    gd2 = spool.tile([48, 48], f32)
                    nc.vector.tensor_mul(gd2[:], G[:, 48:96], eye48[:])
                    nc.vector.tensor_reduce(
                        ssqk2[:], gd2[:], axis=mybir.AxisListType.X,
                        op=mybir.AluOpType.add)

                if upto < 3:
                    continue

                # ---- attention (tiny, partitions 0-47) ----
                with tc.tile_pool(name="psS", bufs=1, space="PSUM") as spp:
                    nk = spool.tile([48, 1], f32)
                    nq = spool.tile([48, 1], f32)
                    invk = spool.tile([48, 1], f32)
                    invq = spool.tile([48, 1], f32)
                    nc.scalar.sqrt(nk[:], ssqk2[:])
                    nc.scalar.sqrt(nq[:], ssqq2[:])
                    nc.vector.tensor_scalar_max(nk[:], nk[:], EPS)
                    nc.vector.tensor_scalar_max(nq[:], nq[:], EPS)
                    nc.vector.reciprocal(invk[:], nk[:])
                    nc.vector.reciprocal(invq[:], nq[:])

                    # logits^T scaled by inv_k (rows = k-channels e)
                    m1 = spool.tile([48, 48], f32)
                    nc.vector.tensor_scalar(
                        m1[:], Gqk[:], invk[:], None,
                        op0=mybir.AluOpType.mult)
                    m1T = spp.tile([48, 48], f32, tag="m1T")
                    nc.tensor.transpose(m1T[:], m1[:], eye48[:])
                    L = spool.tile([48, 48], f32)
                    nc.vector.tensor_scalar(
                        L[:], m1T[:], invq[:], temppc[:],
                        op0=mybir.AluOpType.mult, op1=mybir.AluOpType.mult)
                    nc.vector.tensor_add(L[:], L[:], maskbd[:])
                    nrm = spool.tile([48, 1], f32)
                    nc.vector.tensor_reduce(
                        nrm[:], L[:], axis=mybir.AxisListType.X,
                        op=mybir.AluOpType.max, negate=True)
                    E = spool.tile([48, 48], f32)
                    rowsum = spool.tile([48, 1], f32)
                    nc.scalar.activation(
                        E[:], L[:], mybir.ActivationFunctionType.Exp,
                        bias=nrm[:], scale=1.0, accum_out=rowsum[:])
                    invs = spool.tile([48, 1], f32)
                    nc.vector.reciprocal(invs[:], rowsum[:])
                    attn = spool.tile([48, 48], bf)
                    nc.vector.tensor_scalar(
                        attn[:], E[:], invs[:], None,
                        op0=mybir.AluOpType.mult)

                    # P2^T = attn^T @ proj^T
                    pt_ps = spp.tile([48, 48], f32, tag="ptps")
                    nc.tensor.matmul(pt_ps[:], attn[:], projT[:],
                                     start=True, stop=True)
                    PT = spool.tile([48, 48], bf)
                    nc.vector.tensor_copy(PT[:], pt_ps[:])

                    # phase-2 weight stacks C1 [128, 3*48], CL [48, 48]
                    for dx in range(3):
                        psA = spp.tile([96, 48], f32, tag="psA")
                        for ty in range(2):
                            nc.tensor.matmul(
                                psA[:], vA[:, (dx * 2 + ty) * 96:
                                            (dx * 2 + ty + 1) * 96],
                                PT[:], start=(ty == 0), stop=(ty == 1))
                        nc.vector.tensor_copy(
                            C1[0:96, dx * 48:(dx + 1) * 48], psA[:])
                        psT2 = spp.tile([16, 48], f32, tag="psT2")
                        nc.tensor.matmul(
                            psT2[:], vT2[:, dx * 16:(dx + 1) * 16],
                            PT[:], start=True, stop=True)
                        nc.vector.tensor_copy(
                            C1[96:112, dx * 48:(dx + 1) * 48], psT2[:])
                    for dg in range(3):
                        psL = spp.tile([32, 48], f32, tag="psL")
                        nc.tensor.matmul(
                            psL[:], vTL[:, dg * 32:(dg + 1) * 32],
                            PT[:], start=True, stop=True)
                        nc.vector.tensor_copy(
                            CL[32 * dg:32 * dg + 32, 0:48], psL[:])

                # ---- phase 2: final fused conv + DMA out ----
                with tc.tile_pool(name="psO", bufs=_BUFS["op"],
                                  space="PSUM") as opool:
                    xLt2 = [None]
                    o_sb = None
                    for ci in range(NCH):
                        if ci % CSEG == 0:
                            xLt2[0] = fill_xL(ci // CSEG)
                        o_ps = opool.tile([96, 512], f32)
                        for dx in range(3):
                            nc.tensor.matmul(
                                o_ps[0:48, :], C1[:, dx * 48:(dx + 1) * 48],
                                _w3(x3, ci, dx), start=(dx == 0), stop=False)
                        nc.tensor.matmul(
                            o_ps[0:96, :], CL[:], _wL(xLt2[0], ci % CSEG),
                            start=False, stop=True)
                        if ci % 2 == 0:
                            o_sb = wpool.tile([48, 1024], bf, tag="osb")
                        oslc = o_sb[:, (ci % 2) * 512:(ci % 2 + 1) * 512]
                        if cpeng in ("dve",) or (cpeng == "alt" and ci % 2):
                            nc.vector.tensor_copy(oslc, o_ps[0:48, :])
                        else:
                            nc.scalar.copy(oslc, o_ps[0:48, :])
                        if ci % 2 == 1:
                            nc.sync.dma_start(
                                y[:, (ci - 1) * 512:(ci + 1) * 512], o_sb[:])

    nc.compile()
    return nc


def _host_weights(qkv_w, dw_w, proj_w, temperature):
    # fused qk weights: wfull[t, c, d, o] = qkv_w[o,c] * dw_w[o,0,t,d]
    wfull = np.einsum("oc,otd->tcdo", qkv_w[:96], dw_w[:96, 0]).astype(F32)
    w1 = np.zeros((128, 3, 96), F32)
    w1[0:48] = wfull[0].transpose(0, 1, 2)
    w1[48:96] = wfull[1]
    w1[96:128] = wfull[2, 0:32]
    w1 = w1.reshape(128, 3 * 96).astype(BF16)
    w1 = w1.reshape(128, 3, 96)
    w1[112:128] = 0.0          # ty2 ch16-31 handled by the xL stack
    w1 = w1.reshape(128, 3 * 96)
    wl1 = np.zeros((96, 96), F32)
    for dg in range(3):
        wl1[32 * dg:32 * dg + 32] = wfull[2, 16:48, dg]
    wl1 = wl1.astype(BF16)

    # v-path taps: vw[t, d, e, c] = qkv_w[96+e, c] * dw_w[96+e, 0, t, d]
    vw = np.einsum("ec,etd->tdec", qkv_w[96:], dw_w[96:, 0]).astype(F32)
    vA = np.zeros((48, 6, 96), F32)
    for dx in range(3):
        for ty in range(2):
            for c in range(48):
                vA[:, dx * 2 + ty, ty * 48 + c] = vw[ty, dx, :, c]
    vA = vA.reshape(48, 6 * 96).astype(BF16)
    vT2 = np.zeros((48, 3, 16), F32)
    vTL = np.zeros((48, 3, 32), F32)
    for dx in range(3):
        vT2[:, dx] = vw[2, dx, :, 0:16]
        vTL[:, dx] = vw[2, dx, :, 16:48]
    vT2 = vT2.reshape(48, 3 * 16).astype(BF16)
    vTL = vTL.reshape(48, 3 * 32).astype(BF16)

    projT = proj_w.T.astype(BF16).copy()
    id96 = np.eye(96, dtype=F32).astype(BF16)
    eye48 = np.eye(48, dtype=F32)
    maskbd = np.full((48, 48), -1e9, F32)
    for h in range(HEADS):
        maskbd[h * HD:(h + 1) * HD, h * HD:(h + 1) * HD] = 0.0
    temppc = np.repeat(temperature.reshape(HEADS), HD).reshape(48, 1)
    temppc = temppc.astype(F32)
    return dict(w1=w1, wl1=wl1, vA=vA, vT2=vT2, vTL=vTL, projT=projT,
                id96=id96, eye48=eye48, maskbd=maskbd, temppc=temppc)


def make_in_maps(x, qkv_w, dw_w, proj_w, temperature):
    shared = _host_weights(np.asarray(qkv_w, F32), np.asarray(dw_w, F32),
                           np.asarray(proj_w, F32),
                           np.asarray(temperature, F32))
    xp = np.zeros((B, C, H, PW), F32)
    xp[:, :, :, 1:1 + W] = np.asarray(x, F32).reshape(B, C, H, W)
    xp = xp.reshape(B, C, H * PW).astype(BF16)
    maps = []
    for b in range(B):
        m = dict(shared)
        m["xb"] = xp[b]
        maps.append(m)
    return maps


def kernel(x, qkv_w, dw_w, proj_w, temperature):
    nc = _build()
    in_maps = make_in_maps(x, qkv_w, dw_w, proj_w, temperature)
    res = bass_utils.run_bass_kernel_spmd(nc, in_maps, list(range(B)))
    out = np.stack([res.results[b]["y"].reshape(C, H, W) for b in range(B)])
    return out.astype(np.float32)
